# revision 1
# baseline (speedup 1.0000x reference)
"""Trainium2 Bass kernel for the highway-ensemble module.

Math (per sample b):
    s_n    = clients_logit[n,b,:] @ ensemble_scale + ensemble_bias
    sig_n  = sigmoid(s_n)                    (> 0, so L1 norm == plain sum)
    wn_n   = sig_n / sum_m sig_m
    cal    = (sum_n wn_n * clients_logit[n,b,:]) * logit_scale + logit_bias
    carry  = sigmoid(mean_n(clients_feature[n,b,:]) @ W2 + b2)
    out    = carry * cal + (1 - carry) * new_logit[b,:]

Sharding: data-parallel over the batch dim B=8192 across 8 NeuronCores
(1024 rows each); the client dim N=8 stays local; tiny parameters are
replicated. Each core streams its shard once from HBM -> memory-bound.

The two HWDGE queues are byte-balanced per tile (6.5 MB each) and
ordered so phase A's data lands early: scalar carries the logits, one
feature pair and the output store; sync carries 3 feature pairs +
new_logit.  Feature dots run after the weighted-sum chain, matching
arrival order, so the ACT/DVE serial chains never pile up at stream
end.

Two compiled variants, dispatched at runtime on the parameter values:
  - general: arbitrary ensemble_scale / logit_scale / logit_bias
  - fast:    ensemble_scale==1, logit_scale==1, logit_bias==0 (the
             module's init values). Phase A degenerates to plain row
             sums on the ACT engine, sigmoids run per-client so the
             weighted-sum chain starts as soon as each client's logits
             land, and the calibration stage disappears.
"""

import sys

if "/opt/trn_rl_repo" not in sys.path:
    sys.path.insert(0, "/opt/trn_rl_repo")

from contextlib import ExitStack

import numpy as np

import concourse.bass as bass
import concourse.tile as tile
from concourse import bacc, mybir
from concourse import bass_utils
from concourse.bass_utils import run_bass_kernel_spmd

# Artifact upload targets a remote bucket that this container cannot reach;
# only used on trace runs.
bass_utils.upload_artifacts = lambda tmpdir: tmpdir

N_CORES = 8
N_CLIENTS = 8
B = 8192
C = 1000
F = 2048
PB = 128  # batch rows per SBUF tile (partition dim)

FP32 = mybir.dt.float32
BF16 = mybir.dt.bfloat16
ALU = mybir.AluOpType
ACTFN = mybir.ActivationFunctionType

# Tunable build config; DEFAULT_CFG is what kernel() ships.  Chosen by
# on-hardware A/B: fp=4/lp=10 gave the best min AND best slow-mode time
# (HBM contention with the paired NeuronCore makes runs bimodal).
DEFAULT_CFG = dict(
    feat_bf16=False,   # cast features fp32->bf16 during DMA (SWDGE)
    feat_gpsimd=0,     # how many of the 4 feature pair-dots run on GpSimd
    sig1=False,    # one batched sigmoid over [PB,8] instead of per-client
    act_aux=False,  # run nt2 and the wl-init on ACT instead of DVE
    tail2=False,   # last-2-tiles: logits before ft0 + arrival-ordered dots
    sched="exp2",  # "interleave" | "exp2" emission order (fast path); exp2
                   # (feature dots before the wsum chain, store dispatched
                   # from the tail) won every alternating A/B round on HW
    fp_bufs=4,
    lp_bufs=10,
    np_bufs=3,
    op_bufs=2,
    wk_bufs=3,
)


def build_nc(b_shard: int = B // N_CORES, fast: bool = False, cfg: dict | None = None):
    cfg = {**DEFAULT_CFG, **(cfg or {})}
    FEAT_BF16 = cfg["feat_bf16"]
    nc = bacc.Bacc(
        "TRN2", target_bir_lowering=False, debug=False, num_devices=N_CORES
    )
    cf = nc.dram_tensor(
        "clients_feature", [N_CLIENTS, b_shard, F], FP32, kind="ExternalInput"
    ).ap()
    cl = nc.dram_tensor(
        "clients_logit", [N_CLIENTS, b_shard, C], FP32, kind="ExternalInput"
    ).ap()
    nl = nc.dram_tensor("new_logit", [b_shard, C], FP32, kind="ExternalInput").ap()
    es = nc.dram_tensor("ensemble_scale", [C, 1], FP32, kind="ExternalInput").ap()
    eb = nc.dram_tensor("ensemble_bias", [1], FP32, kind="ExternalInput").ap()
    ls = nc.dram_tensor("logit_scale", [C], FP32, kind="ExternalInput").ap()
    lb = nc.dram_tensor("logit_bias", [C], FP32, kind="ExternalInput").ap()
    w2 = nc.dram_tensor("W2", [F, 1], FP32, kind="ExternalInput").ap()
    b2 = nc.dram_tensor("b2", [1], FP32, kind="ExternalInput").ap()
    out = nc.dram_tensor("out", [b_shard, C], FP32, kind="ExternalOutput").ap()

    ntiles = b_shard // PB

    with tile.TileContext(nc) as tc, ExitStack() as ctx:
        consts = ctx.enter_context(tc.tile_pool(name="consts", bufs=1))
        lp = ctx.enter_context(tc.tile_pool(name="lp", bufs=cfg["lp_bufs"]))
        fp = ctx.enter_context(tc.tile_pool(name="fp", bufs=cfg["fp_bufs"]))
        np_ = ctx.enter_context(tc.tile_pool(name="np", bufs=cfg["np_bufs"]))
        op = ctx.enter_context(tc.tile_pool(name="op", bufs=cfg["op_bufs"]))
        scrp = ctx.enter_context(tc.tile_pool(name="scrp", bufs=1))
        wk = ctx.enter_context(tc.tile_pool(name="wk", bufs=cfg["wk_bufs"]))
        sm = ctx.enter_context(tc.tile_pool(name="sm", bufs=4))

        # Parameters broadcast to all 128 partitions once.
        def bcast(src, cols, tag):
            t = consts.tile([PB, cols], FP32, tag=tag)
            nc.gpsimd.dma_start(out=t, in_=src.unsqueeze(0).to_broadcast([PB, cols]))
            return t

        fdt = BF16 if FEAT_BF16 else FP32
        w2B = consts.tile([PB, F], fdt, tag="w2B")
        nc.gpsimd.dma_start(
            out=w2B, in_=w2[:, 0].unsqueeze(0).to_broadcast([PB, F])
        )
        if not fast:
            esB = bcast(es[:, 0], C, "esB")
            lsB = bcast(ls, C, "lsB")
            lbB = bcast(lb, C, "lbB")
        ebB = bcast(eb, 1, "ebB")
        b2B = bcast(b2, 1, "b2B")
        nb2B = consts.tile([PB, 1], FP32, tag="nb2B")
        nc.vector.tensor_scalar_mul(out=nb2B, in0=b2B, scalar1=-1.0)
        # W2 replicated over the client-pair dim via a stride-0 read AP.
        w2Bp = w2B.unsqueeze(1).to_broadcast([PB, 2, F])

        pipe = []  # deferred-tail states (1-tile software pipeline)
        for it in range(ntiles + 1):
            last = it == ntiles
            # --- deferred tail of the previous tile, emitted FIRST so the
            # tail compute sits at the head of each engine queue ----------
            prev = pipe.pop(0) if pipe else None
            o = None
            if prev is not None:
                dot = sm.tile([PB, 1], FP32, tag="dot")
                nc.vector.tensor_reduce(
                    out=dot, in_=prev["dcols"], axis=mybir.AxisListType.X, op=ALU.add
                )
                carry = sm.tile([PB, 1], FP32, tag="carry")
                nc.scalar.activation(
                    out=carry,
                    in_=dot,
                    func=ACTFN.Sigmoid,
                    bias=b2B[:, 0:1],
                    scale=1.0 / N_CLIENTS,
                )
                o = op.tile([PB, C], FP32, tag="o")
                if fast:
                    # out = (carry*rs)*wl + (1-carry)*new.  1-carry is a
                    # free sigmoid of the negated logit, and the new_logit
                    # term is built on ACT, so the post-stream chain is
                    # just the final stt + store.
                    c1 = sm.tile([PB, 1], FP32, tag="c1")
                    nc.scalar.activation(
                        out=c1,
                        in_=dot,
                        func=ACTFN.Sigmoid,
                        bias=nb2B[:, 0:1],
                        scale=-1.0 / N_CLIENTS,
                    )
                    a = sm.tile([PB, 1], FP32, tag="a")
                    nc.vector.tensor_mul(
                        out=a, in0=carry, in1=prev["rs"][:, 0:1]
                    )
                    nt2 = op.tile([PB, C], FP32, tag="nt2")
                    if cfg["sched"] == "exp2" and not cfg["act_aux"]:
                        nc.vector.tensor_scalar_mul(
                            out=nt2, in0=prev["newt"], scalar1=c1[:, 0:1]
                        )
                    else:
                        nc.scalar.activation(
                            out=nt2,
                            in_=prev["newt"],
                            func=ACTFN.Identity,
                            bias=0.0,
                            scale=c1[:, 0:1],
                        )
                    nc.vector.scalar_tensor_tensor(
                        out=o,
                        in0=prev["wl"],
                        scalar=a[:, 0:1],
                        in1=nt2,
                        op0=ALU.mult,
                        op1=ALU.add,
                    )
                else:
                    d = op.tile([PB, C], FP32, tag="d")
                    nc.vector.scalar_tensor_tensor(
                        out=d, in0=prev["wl"], scalar=prev["rs"][:, 0:1],
                        in1=lsB, op0=ALU.mult, op1=ALU.mult,
                    )
                    nc.vector.tensor_add(out=d, in0=d, in1=lbB)
                    nc.vector.tensor_sub(out=d, in0=d, in1=prev["newt"])
                    # out = (cal - new) * carry + new
                    nc.vector.scalar_tensor_tensor(
                        out=o,
                        in0=d,
                        scalar=carry[:, 0:1],
                        in1=prev["newt"],
                        op0=ALU.mult,
                        op1=ALU.add,
                    )
                if last or cfg["sched"] == "exp2":
                    nc.scalar.dma_start(
                        out=out[prev["b0"] : prev["b0"] + PB, :], in_=o
                    )
                    o = None

            if it < ntiles:
                b0 = it * PB
                tail_mode = cfg["tail2"] and it >= ntiles - 2 and fast

                # --- loads.  Queue rings (6.5 MB each, drain together):
                #   scalar: [ft0, L0, L1, L2, L3, store]
                #   sync:   [ft1, ft2, ft3, newt]
                # Features lead so their dots have early data; the last
                # logit pair lands at ~92% of the period, store last so its
                # wait on `o` never head-of-line delays the loads. --------
                def load_ft(q, eng):
                    ft = fp.tile([PB, 2, F], fdt, tag="ft")
                    eng.dma_start(
                        out=ft,
                        in_=cf[2 * q : 2 * q + 2, b0 : b0 + PB, :].transpose(
                            [1, 0, 2]
                        ),
                    )
                    return ft

                def load_logits():
                    Ls = []
                    for q in range(4):
                        Lp = lp.tile([PB, 2, C], FP32, tag="L")
                        nc.scalar.dma_start(
                            out=Lp,
                            in_=cl[2 * q : 2 * q + 2, b0 : b0 + PB, :].transpose(
                                [1, 0, 2]
                            ),
                        )
                        Ls.append(Lp[:, 0, :])
                        Ls.append(Lp[:, 1, :])
                    return Ls

                fts = [None] * 4
                if tail_mode:
                    # logits lead on the scalar ring so the serial phase-A
                    # chains of the final tiles finish inside their period
                    Ls = load_logits()
                    fts[0] = load_ft(0, nc.gpsimd if FEAT_BF16 else nc.scalar)
                    for q in (1, 2, 3):
                        fts[q] = load_ft(q, nc.gpsimd if FEAT_BF16 else nc.sync)
                else:
                    fts[0] = load_ft(0, nc.gpsimd if FEAT_BF16 else nc.scalar)
                    for q in (1, 2, 3):
                        fts[q] = load_ft(q, nc.gpsimd if FEAT_BF16 else nc.sync)
                    Ls = load_logits()

                newt = np_.tile([PB, C], FP32, tag="new")
                nc.sync.dma_start(out=newt, in_=nl[b0 : b0 + PB, :])
                if o is not None:
                    nc.scalar.dma_start(
                        out=out[prev["b0"] : prev["b0"] + PB, :], in_=o
                    )

                # --- compute, emitted in DMA-arrival order so the in-order
                # DVE never head-of-line blocks: the feature dots fill the
                # sig-gated gaps of the weighted-sum chain. ---------------
                s = sm.tile([PB, N_CLIENTS], FP32, tag="s")
                sig = sm.tile([PB, N_CLIENTS], FP32, tag="sig")
                scr = scrp.tile([PB, C], FP32, tag="scr")
                wl = wk.tile([PB, C], FP32, tag="wl")
                dcols = sm.tile([PB, 4], FP32, tag="dcols")

                def feat_dot(q):
                    # in-place elementwise product; only accum_out is used
                    nc.vector.scalar_tensor_tensor(
                        out=fts[q],
                        in0=fts[q],
                        scalar=1.0,
                        in1=w2Bp,
                        op0=ALU.mult,
                        op1=ALU.mult,
                        accum_out=dcols[:, q : q + 1],
                    )

                if fast:
                    # ensemble_scale == 1: s_n is a plain row sum on ACT.
                    # Per-client sigmoid so the weighted-sum chain starts
                    # as soon as each client's logits land; the chain's
                    # first link runs on ACT to shed DVE load.
                    if tail_mode:
                        feat_dot(1)
                    elif cfg["sched"] == "exp2":
                        for q in range(4):
                            feat_dot(q)
                    else:
                        feat_dot(0)
                        feat_dot(1)
                    if cfg["sig1"]:
                        # batched: 8 back-to-back rowsums, one sigmoid, then
                        # the whole wsum chain (fewer ACT ops + sem edges;
                        # with the exp2 DVE order the chain start is gated
                        # by the feature dots anyway)
                        for n in range(N_CLIENTS):
                            nc.scalar.activation(
                                out=scr,
                                in_=Ls[n],
                                func=ACTFN.Identity,
                                bias=0.0,
                                scale=1.0,
                                accum_out=s[:, n : n + 1],
                            )
                        nc.scalar.activation(
                            out=sig, in_=s, func=ACTFN.Sigmoid,
                            bias=ebB[:, 0:1], scale=1.0,
                        )
                        nc.vector.tensor_scalar_mul(
                            out=wl, in0=Ls[0], scalar1=sig[:, 0:1]
                        )
                        for n in range(1, N_CLIENTS):
                            nc.vector.scalar_tensor_tensor(
                                out=wl, in0=Ls[n], scalar=sig[:, n : n + 1],
                                in1=wl, op0=ALU.mult, op1=ALU.add,
                            )
                    _clients = () if cfg["sig1"] else range(N_CLIENTS)
                    for n in _clients:
                        nc.scalar.activation(
                            out=scr,
                            in_=Ls[n],
                            func=ACTFN.Identity,
                            bias=0.0,
                            scale=1.0,
                            accum_out=s[:, n : n + 1],
                        )
                        nc.scalar.activation(
                            out=sig[:, n : n + 1],
                            in_=s[:, n : n + 1],
                            func=ACTFN.Sigmoid,
                            bias=ebB[:, 0:1],
                            scale=1.0,
                        )
                        if n == 0:
                            if cfg["sched"] == "exp2" and not cfg["act_aux"]:
                                nc.vector.tensor_scalar_mul(
                                    out=wl, in0=Ls[0], scalar1=sig[:, 0:1]
                                )
                            else:
                                nc.scalar.activation(
                                    out=wl,
                                    in_=Ls[0],
                                    func=ACTFN.Identity,
                                    bias=0.0,
                                    scale=sig[:, 0:1],
                                )
                        else:
                            nc.vector.scalar_tensor_tensor(
                                out=wl,
                                in0=Ls[n],
                                scalar=sig[:, n : n + 1],
                                in1=wl,
                                op0=ALU.mult,
                                op1=ALU.add,
                            )
                        if n == 3 and not tail_mode and cfg["sched"] != "exp2":
                            feat_dot(2)
                    # rs = 1 / sum_n sig
                    ssum = sm.tile([PB, 1], FP32, tag="ssum")
                    nc.vector.tensor_reduce(
                        out=ssum, in_=sig, axis=mybir.AxisListType.X, op=ALU.add
                    )
                    rs = sm.tile([PB, 1], FP32, tag="rs")
                    nc.vector.reciprocal(out=rs, in_=ssum)
                    if tail_mode:
                        feat_dot(2)
                        feat_dot(3)
                        feat_dot(0)
                    elif cfg["sched"] != "exp2":
                        feat_dot(3)
                else:
                    for q in range(4):
                        feat_dot(q)
                    for n in range(N_CLIENTS):
                        nc.vector.scalar_tensor_tensor(
                            out=scr,
                            in0=Ls[n],
                            scalar=1.0,
                            in1=esB,
                            op0=ALU.mult,
                            op1=ALU.mult,
                            accum_out=s[:, n : n + 1],
                        )
                    nc.scalar.activation(
                        out=sig, in_=s, func=ACTFN.Sigmoid, bias=ebB[:, 0:1],
                        scale=1.0,
                    )
                    nc.vector.tensor_scalar_mul(out=wl, in0=Ls[0], scalar1=sig[:, 0:1])
                    for n in range(1, N_CLIENTS):
                        nc.vector.scalar_tensor_tensor(
                            out=wl,
                            in0=Ls[n],
                            scalar=sig[:, n : n + 1],
                            in1=wl,
                            op0=ALU.mult,
                            op1=ALU.add,
                        )
                    ssum = sm.tile([PB, 1], FP32, tag="ssum")
                    nc.vector.tensor_reduce(
                        out=ssum, in_=sig, axis=mybir.AxisListType.X, op=ALU.add
                    )
                    rs = sm.tile([PB, 1], FP32, tag="rs")
                    nc.vector.reciprocal(out=rs, in_=ssum)

                pipe.append(dict(b0=b0, dcols=dcols, wl=wl, rs=rs, newt=newt))

    nc.compile()
    return nc


_NC_CACHE = {}


def _get_nc(b_shard, fast, cfg=None):
    key = (b_shard, fast, tuple(sorted(({**DEFAULT_CFG, **(cfg or {})}).items())))
    if key not in _NC_CACHE:
        _NC_CACHE[key] = build_nc(b_shard, fast, cfg)
    return _NC_CACHE[key]


def _run(inputs, trace=False, force_general=False, cfg=None):
    b = int(np.asarray(inputs["new_logit"]).shape[0])
    b_shard = b // N_CORES

    cf = np.ascontiguousarray(np.asarray(inputs["clients_feature"], dtype=np.float32))
    cl = np.ascontiguousarray(np.asarray(inputs["clients_logit"], dtype=np.float32))
    nl = np.ascontiguousarray(np.asarray(inputs["new_logit"], dtype=np.float32))
    rep = {
        k: np.ascontiguousarray(np.asarray(inputs[k], dtype=np.float32))
        for k in (
            "ensemble_scale",
            "ensemble_bias",
            "logit_scale",
            "logit_bias",
            "W2",
            "b2",
        )
    }

    fast = (
        not force_general
        and bool(np.all(rep["ensemble_scale"] == 1.0))
        and bool(np.all(rep["logit_scale"] == 1.0))
        and bool(np.all(rep["logit_bias"] == 0.0))
    )
    nc = _get_nc(b_shard, fast, cfg)

    in_maps = []
    for c in range(N_CORES):
        lo, hi = c * b_shard, (c + 1) * b_shard
        in_maps.append(
            {
                "clients_feature": np.ascontiguousarray(cf[:, lo:hi, :]),
                "clients_logit": np.ascontiguousarray(cl[:, lo:hi, :]),
                "new_logit": np.ascontiguousarray(nl[lo:hi, :]),
                **rep,
            }
        )

    res = run_bass_kernel_spmd(
        nc, in_maps, core_ids=list(range(N_CORES)), trace=trace
    )
    out = np.concatenate([res.results[c]["out"] for c in range(N_CORES)], axis=0)
    return out, res


def kernel(**inputs) -> np.ndarray:
    out, _ = _run(inputs, trace=False)
    return out


def kernel_traced(**inputs):
    """Like kernel() but returns (output, BassKernelResults) with NTFF timing."""
    return _run(inputs, trace=True)


def kernel_traced_cfg(cfg, **inputs):
    """Traced run with a config override (for A/B experiments)."""
    return _run(inputs, trace=True, cfg=cfg)


def kernel_traced_general(**inputs):
    """Force the general (non-specialized) variant, traced."""
    return _run(inputs, trace=True, force_general=True)

